# revision 10
# baseline (speedup 1.0000x reference)
"""CrossAttention3D Trainium2 kernel, 8-way head-sharded.

Strategy: core h computes head h end-to-end:
  - GroupNorm folded into conv weights (stats on device, scale/shift folded
    into the 1x1-conv weight columns and bias).
  - q/k/v 1x1 convs as K=512 matmuls (fp32r).
  - Attention in S^T orientation: S_T[m,n] = k.q, exp on ACT (scale folded),
    P@V with a ones-column appended to v^T so the softmax denominator drops
    out of the same PSUM accumulation.
  - Per-token normalization via reciprocal + partition-broadcast.
  - AllToAll moves head-channels to token-slices; proj + bias + residual per
    token slice on each core; host concatenates the 8 slices.
"""
import sys

sys.path.insert(0, "/opt/trn_rl_repo")

import numpy as np

import concourse.bacc as bacc
import concourse.bass as bass
import concourse.tile as tile
from concourse import mybir
from concourse.bass_utils import run_bass_kernel_spmd
from concourse.masks import make_identity

F32 = mybir.dt.float32
F32R = mybir.dt.float32r
NCORES = 8
C = 512          # channels
NT = 4096        # tokens (T*H*W)
HD = 64          # head dim
G = 8            # groups
P = 128
CT = C // P      # 4 channel tiles
NSUP = 4         # n supers
SUPW = NT // NSUP  # 1024
MCH = NT // P    # 32 m-chunks
EPS = 1e-5
SCALE = HD ** -0.5

_CACHE = {}


def r(ap):
    return ap.bitcast(F32R)


def build_program():
    nc = bacc.Bacc("TRN2", target_bir_lowering=False, debug=False,
                   num_devices=NCORES)

    def din(name, shape):
        return nc.dram_tensor(name, shape, F32, kind="ExternalInput").ap()

    x4 = din("x4", [CT, P, NT])
    c4 = din("c4", [CT, P, NT])
    qwT = din("qwT", [CT, P, HD])
    kwT = din("kwT", [CT, P, HD])
    vwT = din("vwT", [CT, P, HD])
    pwT = din("pwT", [CT, P, C])
    qb = din("qb", [HD, 1])
    kb = din("kb", [HD, 1])
    vb = din("vb", [HD, 1])
    pb = din("pb", [CT, P, 1])
    nqw = din("nqw", [P, CT])
    nqb = din("nqb", [P, CT])
    nkw = din("nkw", [P, CT])
    nkb = din("nkb", [P, CT])
    emat = din("emat", [CT, P, G])
    xs = din("xs", [CT, P, C])
    out_d = nc.dram_tensor("out", [CT, P, C], F32, kind="ExternalOutput").ap()

    with tile.TileContext(nc) as tc:
        with tc.tile_pool(name="wp", bufs=1) as wp, \
             tc.tile_pool(name="qk", bufs=1) as qk, \
             tc.tile_pool(name="sp", bufs=2) as sp, \
             tc.tile_pool(name="dr", bufs=2, space="DRAM") as dr:
            # ---- persistent small tensors ----
            qwT_s = wp.tile([P, CT, HD], F32)
            kwT_s = wp.tile([P, CT, HD], F32)
            vwT_s = wp.tile([P, CT, HD], F32)
            pwT_s = wp.tile([P, CT, C], F32R)
            qb_s = wp.tile([HD, 1], F32)
            kb_s = wp.tile([HD, 1], F32)
            vb_s = wp.tile([HD, 1], F32)
            pb_s = wp.tile([P, CT], F32)
            nqw_s = wp.tile([P, CT], F32)
            nqb_s = wp.tile([P, CT], F32)
            nkw_s = wp.tile([P, CT], F32)
            nkb_s = wp.tile([P, CT], F32)
            em_s = wp.tile([P, CT, G], F32)
            xs_s = wp.tile([P, CT, C], F32)
            ident = wp.tile([P, P], F32)
            eps_s = wp.tile([G, 1], F32)
            kbe = wp.tile([HD, 1], F32)
            vbe = wp.tile([HD, 1], F32)
            qbe = wp.tile([HD, 1], F32)
            a2a_in = dr.tile([NCORES, HD, C], F32, tag="a2ain")
            a2a_out = dr.tile([NCORES, HD, C], F32, tag="a2aout")

            for t in range(CT):
                nc.sync.dma_start(qwT_s[:, t, :], qwT[t])
                nc.sync.dma_start(kwT_s[:, t, :], kwT[t])
                nc.sync.dma_start(vwT_s[:, t, :], vwT[t])
                nc.sync.dma_start(pb_s[:, t : t + 1], pb[t])
                nc.sync.dma_start(xs_s[:, t, :], xs[t])
                nc.sync.dma_start(em_s[:, t, :], emat[t])
            nc.sync.dma_start(qb_s[:], qb[:, :])
            nc.sync.dma_start(kb_s[:], kb[:, :])
            nc.sync.dma_start(vb_s[:], vb[:, :])
            nc.sync.dma_start(nqw_s[:], nqw[:, :])
            nc.sync.dma_start(nqb_s[:], nqb[:, :])
            nc.sync.dma_start(nkw_s[:], nkw[:, :])
            nc.sync.dma_start(nkb_s[:], nkb[:, :])
            nc.vector.memset(eps_s[:], EPS)
            make_identity(nc, ident[:])
            for t in range(CT):
                pst = sp.tile([P, C], F32, tag="pst")
                nc.sync.dma_start(pst[:], pwT[t])
                nc.vector.tensor_copy(pwT_s[:, t, :], pst[:])

            q_sb = qk.tile([HD, NT], F32R)
            k_sb = qk.tile([HD, NT], F32R)
            vt_sb = qk.tile([P, MCH, HD + 1], F32R)
            ones_st = wp.tile([P, MCH, 1], F32)
            nc.vector.memset(ones_st[:], 1.0)
            nc.vector.tensor_copy(vt_sb[:, :, HD : HD + 1], ones_st[:])

            stat_dram = dr.tile([4 * G], F32, tag="stat")
            rdram = dr.tile([NSUP, SUPW], F32, tag="rd")

            def stats_and_fold(src_tiles, nw_t, nb_t, gs_pool, which):
                """compute per-group mu/rstd of src, return (a, beta) (P,CT)."""
                gp = gs_pool.tile([G, 2], F32, tag="gs")
                for t in range(CT):
                    st = sp.tile([P, 8, 6], F32, tag="bnst")
                    for ch in range(8):
                        nc.vector.bn_stats(
                            out=st[:, ch, :],
                            in_=src_tiles[t][:, ch * 512 : (ch + 1) * 512].bitcast(F32),
                        )
                    mv = sp.tile([P, 2], F32, tag="mv")
                    nc.vector.bn_aggr(out=mv[:], in_=st[:])
                    ss = sp.tile([P, 2], F32, tag="ss")
                    nc.vector.tensor_copy(ss[:, 0:1], mv[:, 0:1])
                    m2 = sp.tile([P, 1], F32, tag="m2")
                    nc.vector.tensor_mul(m2[:], mv[:, 0:1], mv[:, 0:1])
                    nc.vector.tensor_add(ss[:, 1:2], mv[:, 1:2], m2[:])
                    nc.tensor.matmul(gp[:], em_s[:, t, :], ss[:],
                                     start=(t == 0), stop=(t == CT - 1))
                gs = sp.tile([G, 2], F32, tag="gsb")
                nc.vector.tensor_copy(gs[:], gp[:])
                mu = gs[:, 0:1]
                var = sp.tile([G, 1], F32, tag="var")
                nc.vector.tensor_mul(var[:], gs[:, 0:1], gs[:, 0:1])
                nc.vector.tensor_sub(var[:], gs[:, 1:2], var[:])
                nc.scalar.activation(out=var[:], in_=var[:],
                                     func=mybir.ActivationFunctionType.Sqrt,
                                     bias=eps_s[:], scale=1.0)
                rstd = sp.tile([G, 1], F32, tag="rstd")
                nc.vector.reciprocal(rstd[:], var[:])
                off = which * 2 * G
                nc.sync.dma_start(stat_dram[off : off + G], rstd[:, 0])
                nc.sync.dma_start(stat_dram[off + G : off + 2 * G], mu[:, 0:1])
                rb = sp.tile([P, CT], F32, tag="rb")
                mb = sp.tile([P, CT], F32, tag="mb")
                for t in range(CT):
                    src_r = bass.AP(tensor=stat_dram.tensor,
                                    offset=stat_dram.offset + off + 2 * t,
                                    ap=[[1, 2], [0, HD]])
                    nc.gpsimd.dma_start(out=rb[:, t : t + 1], in_=src_r)
                    src_m = bass.AP(tensor=stat_dram.tensor,
                                    offset=stat_dram.offset + off + G + 2 * t,
                                    ap=[[1, 2], [0, HD]])
                    nc.gpsimd.dma_start(out=mb[:, t : t + 1], in_=src_m)
                a = sp.tile([P, CT], F32, tag=f"a{which}")
                beta = sp.tile([P, CT], F32, tag=f"beta{which}")
                nc.vector.tensor_mul(a[:], rb[:], nw_t[:])
                nc.vector.tensor_mul(beta[:], mb[:], a[:])
                nc.vector.tensor_sub(beta[:], nb_t[:], beta[:])
                return a, beta

            def fold_bias(wT_t, beta, b_in, b_out, ps_pool):
                bp = ps_pool.tile([HD, 1], F32, tag="bias")
                for t in range(CT):
                    nc.tensor.matmul(bp[:], wT_t[:, t, :], beta[:, t : t + 1],
                                     start=(t == 0), stop=(t == CT - 1))
                nc.vector.tensor_add(b_out[:], bp[:], b_in[:])

            def conv(wT_t, src_tiles, b_eff, dst, ps_pool):
                for j in range(NT // 512):
                    cp = ps_pool.tile([HD, 512], F32, tag="conv")
                    for t in range(CT):
                        nc.tensor.matmul(
                            cp[:], wT_t[:, t, :],
                            src_tiles[t][:, j * 512 : (j + 1) * 512],
                            start=(t == 0), stop=(t == CT - 1))
                    nc.vector.tensor_scalar_add(
                        dst[:, j * 512 : (j + 1) * 512], cp[:], b_eff[:])

            # ================= preamble =================
            with tc.tile_pool(name="vv", bufs=1) as vv, \
                 tc.tile_pool(name="pp0", bufs=1, space="PSUM") as pp0, \
                 tc.tile_pool(name="ppc", bufs=2, space="PSUM") as ppc, \
                 tc.tile_pool(name="ppt", bufs=2, space="PSUM") as ppt:
                cx_cm = tc.tile_pool(name="cx", bufs=1)
                cx = cx_cm.__enter__()
                ctx_t = [cx.tile([P, NT], F32R, tag=f"c{t}", name=f"ctx{t}") for t in range(CT)]
                for t in range(CT):
                    for ch in range(4):
                        cstg = sp.tile([P, SUPW], F32, tag="stg", bufs=3,
                                       name=f"cstg{t}{ch}")
                        nc.sync.dma_start(
                            cstg[:], c4[t][:, ch * SUPW:(ch + 1) * SUPW])
                        nc.vector.tensor_copy(
                            ctx_t[t][:, ch * SUPW:(ch + 1) * SUPW], cstg[:])

                a_c, beta_c = stats_and_fold(ctx_t, nkw_s, nkb_s, pp0, 0)
                kwTs = sp.tile([P, CT, HD], F32R, tag="kwTs", bufs=1)
                vwTs = sp.tile([P, CT, HD], F32R, tag="vwTs", bufs=1)
                for t in range(CT):
                    nc.vector.tensor_scalar_mul(kwTs[:, t, :], kwT_s[:, t, :],
                                                a_c[:, t : t + 1])
                    nc.vector.tensor_scalar_mul(vwTs[:, t, :], vwT_s[:, t, :],
                                                a_c[:, t : t + 1])
                fold_bias(kwT_s, beta_c, kb_s, kbe, pp0)
                fold_bias(vwT_s, beta_c, vb_s, vbe, pp0)

                v_sb = vv.tile([HD, NT], F32)
                conv(kwTs, ctx_t, kbe, k_sb, ppc)
                conv(vwTs, ctx_t, vbe, v_sb, ppc)
                cx_cm.__exit__(None, None, None)
                xx_cm = tc.tile_pool(name="xx", bufs=1)
                xx = xx_cm.__enter__()
                x_t = [xx.tile([P, NT], F32R, tag=f"x{t}", name=f"xt{t}") for t in range(CT)]
                for t in range(CT):
                    for ch in range(4):
                        xstg = sp.tile([P, SUPW], F32, tag="stg", bufs=3,
                                       name=f"xstg{t}{ch}")
                        nc.sync.dma_start(
                            xstg[:], x4[t][:, ch * SUPW:(ch + 1) * SUPW])
                        nc.vector.tensor_copy(
                            x_t[t][:, ch * SUPW:(ch + 1) * SUPW], xstg[:])

                # v transpose -> vt_sb[:, i, 0:HD]
                for i in range(MCH):
                    tp = ppt.tile([P, HD], F32, tag="tp")
                    nc.tensor.transpose(tp[:], v_sb[:, i * P : (i + 1) * P],
                                        ident[0:HD, 0:HD])
                    nc.vector.tensor_copy(vt_sb[:, i, 0:HD], tp[:])

                a_x, beta_x = stats_and_fold(x_t, nqw_s, nqb_s, pp0, 1)
                qwTs = sp.tile([P, CT, HD], F32R, tag="qwTs", bufs=1)
                for t in range(CT):
                    nc.vector.tensor_scalar_mul(qwTs[:, t, :], qwT_s[:, t, :],
                                                a_x[:, t : t + 1])
                fold_bias(qwT_s, beta_x, qb_s, qbe, pp0)

                # warm the exp table while q conv runs
                dummy = sp.tile([1, 2], F32, tag="dum")
                nc.vector.memset(dummy[:], 0.0)
                nc.scalar.activation(out=dummy[:], in_=dummy[:],
                                     func=mybir.ActivationFunctionType.Exp,
                                     scale=1.0)

                conv(qwTs, x_t, qbe, q_sb, ppc)
                xx_cm.__exit__(None, None, None)

            # ================= attention =================
            with tc.tile_pool(name="pps", bufs=2, space="PSUM") as pps, \
                 tc.tile_pool(name="ppu", bufs=2, space="PSUM") as ppu, \
                 tc.tile_pool(name="pexp", bufs=3) as pexp, \
                 tc.tile_pool(name="uflush", bufs=2) as ufl:
                for s in range(NSUP):
                    u_ps = ppu.tile([HD + 1, SUPW], F32, tag="u")
                    for m in range(MCH):
                        s_ps = pps.tile([P, SUPW], F32, tag="s")
                        for jj in range(2):
                            nsl = slice(s * SUPW + jj * 512,
                                        s * SUPW + (jj + 1) * 512)
                            nc.tensor.matmul(
                                s_ps[:, jj * 512 : (jj + 1) * 512],
                                k_sb[:, m * P : (m + 1) * P],
                                q_sb[:, nsl],
                                start=True, stop=True)
                        p_sb = pexp.tile([P, SUPW], F32R, tag="p")
                        nc.scalar.activation(out=p_sb[:], in_=s_ps[:],
                                             func=mybir.ActivationFunctionType.Exp,
                                             scale=SCALE)
                        for jj in range(2):
                            nc.tensor.matmul(
                                u_ps[:, jj * 512 : (jj + 1) * 512],
                                vt_sb[:, m, :],
                                p_sb[:, jj * 512 : (jj + 1) * 512],
                                start=(m == 0), stop=(m == MCH - 1))
                    # flush + normalize this super
                    u_sb = ufl.tile([HD + 1, SUPW], F32, tag="us")
                    nc.vector.tensor_copy(u_sb[:], u_ps[:])
                    rcp = ufl.tile([1, SUPW], F32, tag="rcp")
                    nc.vector.reciprocal(rcp[:], u_sb[HD : HD + 1, :])
                    rb = ufl.tile([HD, SUPW], F32, tag="rbb")
                    nc.sync.dma_start(rdram[s : s + 1, :], rcp[:])
                    src = bass.AP(tensor=rdram.tensor,
                                  offset=rdram.offset + s * SUPW,
                                  ap=[[0, HD], [1, SUPW]])
                    nc.gpsimd.dma_start(out=rb[:], in_=src)
                    for jj in range(2):
                        u2 = ufl.tile([HD, 512], F32, tag="u2")
                        nc.vector.tensor_mul(u2[:],
                                             u_sb[0:HD, jj * 512 : (jj + 1) * 512],
                                             rb[:, jj * 512 : (jj + 1) * 512])
                        nc.sync.dma_start(a2a_in[2 * s + jj], u2[:])

            # ================= all-to-all + proj =================
            nc.gpsimd.collective_compute(
                "AllToAll", mybir.AluOpType.bypass,
                replica_groups=[list(range(NCORES))],
                ins=[a2a_in.opt()], outs=[a2a_out.opt()])

            with tc.tile_pool(name="ppj", bufs=2, space="PSUM") as ppj, \
                 tc.tile_pool(name="at", bufs=1) as atp:
                at_t = [atp.tile([P, C], F32R, tag=f"at{t}", name=f"att{t}") for t in range(CT)]
                for t in range(CT):
                    ast = sp.tile([P, C], F32, tag="ast", bufs=2, name=f"ast{t}")
                    nc.sync.dma_start(
                        ast[:],
                        a2a_out[2 * t : 2 * t + 2].rearrange("a b c -> (a b) c"))
                    nc.vector.tensor_copy(at_t[t][:], ast[:])
                for t in range(CT):
                    pj = ppj.tile([P, C], F32, tag="pj")
                    for kk in range(CT):
                        nc.tensor.matmul(pj[:],
                                         pwT_s[:, kk, t * P : (t + 1) * P],
                                         at_t[kk][:],
                                         start=(kk == 0), stop=(kk == CT - 1))
                    o_sb = sp.tile([P, C], F32, tag="osb")
                    nc.vector.scalar_tensor_tensor(
                        out=o_sb[:], in0=pj[:], scalar=pb_s[:, t : t + 1],
                        in1=xs_s[:, t, :],
                        op0=mybir.AluOpType.add, op1=mybir.AluOpType.add)
                    nc.sync.dma_start(out_d[t], o_sb[:])

    nc.compile()
    return nc


def _prep_inputs(x, context, norm_q_w, norm_q_b, norm_kv_w, norm_kv_b,
                 q_w, q_b, kv_w, kv_b, proj_w, proj_b):
    xf = np.ascontiguousarray(np.asarray(x, np.float32).reshape(C, NT))
    cf = np.ascontiguousarray(np.asarray(context, np.float32).reshape(C, NT))
    x4 = xf.reshape(CT, P, NT)
    c4 = cf.reshape(CT, P, NT)
    pwT = np.ascontiguousarray(np.asarray(proj_w, np.float32).T).reshape(CT, P, C)
    pb = np.asarray(proj_b, np.float32).reshape(CT, P, 1)
    emat = np.zeros((CT, P, G), np.float32)
    for t in range(CT):
        for p in range(P):
            g = (t * P + p) // HD
            emat[t, p, g] = 1.0 / HD
    nqw = np.ascontiguousarray(np.asarray(norm_q_w, np.float32).reshape(CT, P).T)
    nqb = np.ascontiguousarray(np.asarray(norm_q_b, np.float32).reshape(CT, P).T)
    nkw = np.ascontiguousarray(np.asarray(norm_kv_w, np.float32).reshape(CT, P).T)
    nkb = np.ascontiguousarray(np.asarray(norm_kv_b, np.float32).reshape(CT, P).T)
    q_w = np.asarray(q_w, np.float32)
    kv_w = np.asarray(kv_w, np.float32)
    q_b = np.asarray(q_b, np.float32)
    kv_b = np.asarray(kv_b, np.float32)
    in_maps = []
    for h in range(NCORES):
        hs = HD * h
        in_maps.append({
            "x4": x4, "c4": c4,
            "qwT": np.ascontiguousarray(q_w[hs:hs + HD, :].T).reshape(CT, P, HD),
            "kwT": np.ascontiguousarray(kv_w[hs:hs + HD, :].T).reshape(CT, P, HD),
            "vwT": np.ascontiguousarray(kv_w[C + hs:C + hs + HD, :].T).reshape(CT, P, HD),
            "pwT": pwT,
            "qb": q_b[hs:hs + HD].reshape(HD, 1),
            "kb": kv_b[hs:hs + HD].reshape(HD, 1),
            "vb": kv_b[C + hs:C + hs + HD].reshape(HD, 1),
            "pb": pb, "nqw": nqw, "nqb": nqb, "nkw": nkw, "nkb": nkb,
            "emat": emat,
            "xs": np.ascontiguousarray(xf[:, h * C:(h + 1) * C]).reshape(CT, P, C),
        })
    return in_maps


def kernel(**inputs):
    if "nc" not in _CACHE:
        _CACHE["nc"] = build_program()
    nc = _CACHE["nc"]
    in_maps = _prep_inputs(**inputs)
    res = run_bass_kernel_spmd(nc, in_maps, list(range(NCORES)))
    _CACHE["last_results"] = res
    full = np.empty((C, NT), np.float32)
    for i in range(NCORES):
        full[:, i * C:(i + 1) * C] = res.results[i]["out"].reshape(C, C)
    return full.reshape(1, C, 4, 32, 32)
